# revision 17
# baseline (speedup 1.0000x reference)
"""Bass/Trainium2 kernel for batched 3D FFT circular convolution.

Reference computes y = Re(IFFT3(FFT3(x) * FFT3(w))) with net scaling
circular_conv3d(x, w) / sqrt(N); x: (16, 32, 128, 128) f32, w: (32, 128, 128).

Strategy (data parallel over batch, 8 cores x 2 samples):
- Pack two real samples as one complex volume z = x0 + i*x1; y0 = Re, y1 = Im.
- All FFTs as DFT matmuls in bf16 (inputs pre-rounded to bf16 on host).
- Transposes are FUSED into the DFT matmuls: the DFT matrix is symmetric, so
  making the DATA the stationary operand computes data^T @ F = (F @ data)^T --
  the transform output lands transposed (next axis on partitions) for free.
- Complex arithmetic pairs both component matrices in ONE 256-wide moving
  operand [F_R | F_I]; each data block needs only 2 matmuls (2 LDW).
- Layout permutations ride in strided stationary APs (single-strided slices,
  plus 32-col tile_position strips for stage S2); PSUM evictions contiguous.
- Host pre-transposes x,w to (d2,d1,d3) and un-transposes y so every DMA run
  is >= 1KB contiguous (otherwise DMA is descriptor-rate bound).
- W~ = FFT3(w)/(N*sqrt(N)) computed on-device per core (replicated; an
  AllGather-sharded variant measured WORSE: cross-core start skew ~20us makes
  any mid-kernel collective sync a net loss).
- W-chain stage groups are interleaved one stage ahead of the z-chain so the
  PE has independent work at the z-chain's all-to-all stage boundaries.

Stage layouts (partition | free):
  L0   [d2 | d1,d3]                  f = d1*128 + d3
  S1   fused FFT d2  -> [d3 | d1,k2] f = d1*128 + k2
  S2   fused FFT d3 (4 col-strips per block j, strided 32-col stationary)
                     -> [(k2q,d1) | j,k3]  f = j*128 + k3   (k2 = 4j+k2q)
  S3   BD FFT d1 (weight-stationary) -> [(k2q,k1) | j,k3]
  M    V = Z * W~   (DVE, bf16 2x mode)
  S4   fused BD IFFT d1 -> [k3 | 32*k2 + d1]
  S5   fused IFFT k3 (single-strided stationary, stride 32)
                     -> [k2 | d1,d3]  f = d1*128 + d3
  S6   IFFT k2 (weight-stationary) -> [d2 | d1,d3] -> DMA out
"""

import numpy as np
import ml_dtypes

BF = ml_dtypes.bfloat16

D1, D2, D3 = 32, 128, 128
NTOT = D1 * D2 * D3
FREE = D1 * D3  # 4096
B = 16
NCORES = 8

# paired (128x256) const slots
PF_F2, PI_F2, PF_F2s, PR_F2, PR_BDq, INV_F2_I, INV_BD_I = range(7)
NPAIRS_TOT = 7
# single 128-wide consts for weight-stationary stages
S_F2R, S_F2I, S_F2In, S_BDR, S_BDI, S_BDIn = range(6)
NSNG = 6


def _consts_np():
    k = np.arange(128)
    F2 = np.exp(-2j * np.pi * np.outer(k, k) / 128)
    k1 = np.arange(32)
    F1 = np.exp(-2j * np.pi * np.outer(k1, k1) / 32)
    BD = np.zeros((128, 128), complex)
    for g in range(4):
        BD[32 * g:32 * g + 32, 32 * g:32 * g + 32] = F1
    alpha = 1.0 / (NTOT * np.sqrt(np.float64(NTOT)))
    F2R, F2I = F2.real, F2.imag
    BDR, BDI = BD.real, BD.imag
    pairs = [
        np.concatenate([F2R, F2I], axis=1),             # PF_F2   (fwd, sR)
        np.concatenate([-F2I, F2R], axis=1),            # PI_F2   (fwd, sI)
        np.concatenate([F2R * alpha, F2I * alpha], 1),  # PF_F2s  (fwd, sR)
        np.concatenate([F2R, -F2I], axis=1),            # PR_F2   (inv, sR)
        np.concatenate([BDR, -BDI], axis=1),            # PR_BDq  (inv, sR)
        np.concatenate([F2I, F2R], axis=1),             # INV_F2_I (inv, sI)
        np.concatenate([BDI, BDR], axis=1),             # INV_BD_I (inv, sI)
    ]
    singles = [F2R, F2I, -F2I, BDR, BDI, -BDI]
    mats = np.concatenate([np.concatenate(pairs, axis=1),
                           np.concatenate(singles, axis=1)], axis=1)
    return np.ascontiguousarray(mats, dtype=np.float32).astype(BF)


def _build_program():
    import concourse.mybir as mybir
    import concourse.tile as tile
    from concourse import bacc

    f32 = mybir.dt.float32
    bf16 = mybir.dt.bfloat16

    nc = bacc.Bacc("TRN2")
    # inputs pre-transposed on host to (d2, d1, d3)
    x0_d = nc.dram_tensor("x0", (D2, D1, D3), bf16, kind="ExternalInput")
    x1_d = nc.dram_tensor("x1", (D2, D1, D3), bf16, kind="ExternalInput")
    w_d = nc.dram_tensor("w", (D2, D1, D3), bf16, kind="ExternalInput")
    CW = NPAIRS_TOT * 256 + NSNG * 128
    c_d = nc.dram_tensor("consts", (128, CW), bf16, kind="ExternalInput")
    y0_d = nc.dram_tensor("y0", (D2, D1, D3), f32, kind="ExternalOutput")
    y1_d = nc.dram_tensor("y1", (D2, D1, D3), f32, kind="ExternalOutput")

    with tile.TileContext(nc) as tc:
        with (
            tc.tile_pool(name="sb", bufs=1) as sb,
            tc.tile_pool(name="tp", bufs=2) as tp,
            tc.tile_pool(name="ps", bufs=2, space="PSUM") as ps,
        ):
            consts = sb.tile([128, CW], bf16, name="consts")
            nc.sync.dma_start(out=consts, in_=c_d.ap())

            def P2(i):
                return consts[:, 256 * i:256 * (i + 1)]

            def S1m(i):
                o = NPAIRS_TOT * 256
                return consts[:, o + 128 * i:o + 128 * (i + 1)]

            def vol(name, n=2, dt=bf16, cols=FREE):
                return [sb.tile([128, cols], dt, name=f"{name}{c}")
                        for c in range(n)]

            zA = vol("zA")
            zB = vol("zB")
            VV = vol("VV")
            wA = vol("wA", 1)
            wB = vol("wB")
            wC = vol("wC")
            WT = vol("WT")
            yst = vol("yst", 2, f32)

            # w first (gates the W chain = earliest PE work), then x halves
            nc.sync.dma_start(
                out=wA[0].rearrange("p (a c) -> p a c", a=D1),
                in_=w_d.ap())
            for t in range(2):
                for comp, src in ((0, x0_d), (1, x1_d)):
                    nc.sync.dma_start(
                        out=zA[comp][:, 2048 * t:2048 * (t + 1)].rearrange(
                            "p (a c) -> p a c", a=16),
                        in_=src.ap()[:, 16 * t:16 * (t + 1), :])

            ectr = [0]

            def evict(dst, src):
                # a group's two evictions must land on DIFFERENT engines so
                # the PSUM slot frees after ~one copy latency; alternate which
                # engine leads so totals stay balanced
                lead_v = (ectr[0] // 2) % 2 == 0
                use_v = (ectr[0] % 2 == 0) == lead_v
                if use_v:
                    nc.vector.tensor_copy(dst, src)
                else:
                    nc.scalar.copy(dst, src)
                ectr[0] += 1

            def lhs_for(src, b, stat):
                if stat == "contig":
                    return src[:, 128 * b:128 * (b + 1)]
                # "stride32": f = 32*k2 + d1 -> fixed d1=b, k2 stride 32
                v = src.rearrange("p (k2 d1) -> p k2 d1", k2=128, d1=32)
                return v[:, :, b:b + 1]

            def fused_group(dsts, srcs, pairR, pairI, g, stat="contig",
                            real_in=False):
                """one 8-block psum group of a fused stage."""
                pt = ps.tile([128, 2048], f32, name="pt", tag="ps")
                for q in range(8):
                    b = 8 * g + q
                    o = slice(256 * q, 256 * (q + 1))
                    st = (q % 2 == 0)
                    sp = (q % 2 == 1)
                    if stat == "strips":
                        v0 = srcs[0].rearrange("p (d1 k2) -> p k2 d1",
                                               d1=32, k2=128)
                        v1 = srcs[1].rearrange("p (d1 k2) -> p k2 d1",
                                               d1=32, k2=128)
                        # all 4 R-pass strips first (distinct col_grps run
                        # concurrently), then the 4 accumulating I-pass strips
                        for s in range(4):
                            nc.tensor.matmul(
                                pt[32 * s:32 * (s + 1), o],
                                v0[:, 4 * b + s, :], P2(pairR),
                                start=st, stop=False,
                                tile_position=(0, 32 * s),
                                skip_group_check=True)
                        for s in range(4):
                            nc.tensor.matmul(
                                pt[32 * s:32 * (s + 1), o],
                                v1[:, 4 * b + s, :], P2(pairI),
                                start=False, stop=sp,
                                tile_position=(0, 32 * s),
                                skip_group_check=True)
                    elif real_in:
                        nc.tensor.matmul(pt[:, o], lhs_for(srcs[0], b, stat),
                                         P2(pairR), start=st, stop=sp,
                                         skip_group_check=True)
                    else:
                        nc.tensor.matmul(pt[:, o], lhs_for(srcs[0], b, stat),
                                         P2(pairR), start=st, stop=False,
                                         skip_group_check=True)
                        nc.tensor.matmul(pt[:, o], lhs_for(srcs[1], b, stat),
                                         P2(pairI), start=False, stop=sp,
                                         skip_group_check=True)
                pv = pt.rearrange("p (q c f) -> p c q f", q=8, c=2)
                sl = slice(1024 * g, 1024 * (g + 1))
                dv0 = dsts[0][:, sl].rearrange("p (q f) -> p q f", q=8)
                dv1 = dsts[1][:, sl].rearrange("p (q f) -> p q f", q=8)
                evict(dv0, pv[:, 0])
                evict(dv1, pv[:, 1])

            def std_group(dsts, srcs, mats, t):
                """one 2-chunk (1024-col) psum group of a std stage."""
                mA, mB, mC = mats
                pt = ps.tile([128, 2048], f32, name="pt", tag="ps")
                for h in range(2):
                    s = slice(1024 * t + 512 * h, 1024 * t + 512 * (h + 1))
                    oR = slice(512 * h, 512 * (h + 1))
                    oI = slice(1024 + 512 * h, 1024 + 512 * (h + 1))
                    nc.tensor.matmul(pt[:, oR], S1m(mA), srcs[0][:, s],
                                     start=True, stop=False)
                    nc.tensor.matmul(pt[:, oI], S1m(mC), srcs[0][:, s],
                                     start=True, stop=False)
                    nc.tensor.matmul(pt[:, oR], S1m(mB), srcs[1][:, s],
                                     start=False, stop=True)
                    nc.tensor.matmul(pt[:, oI], S1m(mA), srcs[1][:, s],
                                     start=False, stop=True)
                sl = slice(1024 * t, 1024 * (t + 1))
                evict(dsts[0][:, sl], pt[:, :1024])
                evict(dsts[1][:, sl], pt[:, 1024:])

            FWD_BD = (S_BDR, S_BDIn, S_BDI)
            INV_F2s = (S_F2R, S_F2I, S_F2In)

            def mult_q(qq):
                s = slice(1024 * qq, 1024 * (qq + 1))
                t1 = tp.tile([128, 1024], bf16, name="t1", tag="t1")
                t2 = tp.tile([128, 1024], bf16, name="t2", tag="t2")
                nc.vector.tensor_tensor(t1, zB[0][:, s], WT[0][:, s],
                                        op=mybir.AluOpType.mult)
                nc.vector.tensor_tensor(t2, zB[1][:, s], WT[1][:, s],
                                        op=mybir.AluOpType.mult)
                nc.vector.tensor_tensor(VV[0][:, s], t1, t2,
                                        op=mybir.AluOpType.subtract)
                t3 = tp.tile([128, 1024], bf16, name="t3", tag="t1")
                t4 = tp.tile([128, 1024], bf16, name="t4", tag="t2")
                nc.vector.tensor_tensor(t3, zB[0][:, s], WT[1][:, s],
                                        op=mybir.AluOpType.mult)
                nc.vector.tensor_tensor(t4, zB[1][:, s], WT[0][:, s],
                                        op=mybir.AluOpType.mult)
                nc.vector.tensor_tensor(VV[1][:, s], t3, t4,
                                        op=mybir.AluOpType.add)

            # ---- schedule: W chain one stage ahead, interleaved with z ----
            for g in range(4):
                fused_group(wB, wA, PF_F2s, None, g, real_in=True)   # S1w
            for g in range(4):
                fused_group(zB, zA, PF_F2, PI_F2, g)                 # S1 g
                fused_group(wC, wB, PF_F2, PI_F2, g, stat="strips")  # S2w g
            for g in range(4):
                std_group(WT, wC, FWD_BD, g)                         # S3w g
                fused_group(zA, zB, PF_F2, PI_F2, g, stat="strips")  # S2 g
            # S3 chunk t -> M quarter t -> S4 group t pipeline
            std_group(zB, zA, FWD_BD, 0)                             # S3 t0
            mult_q(0)
            std_group(zB, zA, FWD_BD, 1)
            mult_q(1)
            fused_group(zA, VV, PR_BDq, INV_BD_I, 0)                 # S4 g0
            std_group(zB, zA, FWD_BD, 2)
            mult_q(2)
            fused_group(zA, VV, PR_BDq, INV_BD_I, 1)
            std_group(zB, zA, FWD_BD, 3)
            mult_q(3)
            fused_group(zA, VV, PR_BDq, INV_BD_I, 2)
            fused_group(zA, VV, PR_BDq, INV_BD_I, 3)                 # S4 g3
            for g in range(4):
                fused_group(zB, zA, PR_F2, INV_F2_I, g, stat="stride32")  # S5
            for g in range(4):
                std_group(yst, zB, INV_F2s, g)                       # S6
                for comp, dst in ((0, y0_d), (1, y1_d)):
                    nc.sync.dma_start(
                        out=dst.ap()[:, 8 * g:8 * (g + 1), :],
                        in_=yst[comp][:, 1024 * g:1024 * (g + 1)].rearrange(
                            "p (a c) -> p a c", a=8))
    return nc


_CACHE = {}


def _get_program():
    if "nc" not in _CACHE:
        nc = _build_program()
        try:
            if not nc.is_finalized():
                nc.finalize()
        except AttributeError:
            nc.finalize()
        _CACHE["nc"] = nc
    return _CACHE["nc"]


def _run(x, w_real, **kw):
    from concourse.bass_utils import run_bass_kernel_spmd

    nc = _get_program()
    consts = _consts_np()
    xT = np.ascontiguousarray(
        np.asarray(x, dtype=np.float32).transpose(0, 2, 1, 3)).astype(BF)
    wT = np.ascontiguousarray(
        np.asarray(w_real, dtype=np.float32).transpose(1, 0, 2)).astype(BF)
    in_maps = []
    for c in range(NCORES):
        in_maps.append({
            "x0": xT[2 * c],
            "x1": xT[2 * c + 1],
            "w": wT,
            "consts": consts,
        })
    res = run_bass_kernel_spmd(nc, in_maps, core_ids=list(range(NCORES)), **kw)
    out = np.empty((B, D1, D2, D3), dtype=np.float32)
    for c in range(NCORES):
        out[2 * c] = res.results[c]["y0"].transpose(1, 0, 2)
        out[2 * c + 1] = res.results[c]["y1"].transpose(1, 0, 2)
    return out, res


def kernel(x: np.ndarray, w_real: np.ndarray) -> np.ndarray:
    return _run(x, w_real)[0]


def kernel_traced(x: np.ndarray, w_real: np.ndarray):
    return _run(x, w_real, trace=True)
